# revision 1
# baseline (speedup 1.0000x reference)
"""KAN layer (Chebyshev order-7 on tanh(x)) as a Bass/Tile TRN2 kernel.

Math: out[b,o] = sum_{i,k} T_k(tanh(x[b,i])) * W[o,i,k] + bias[o],  k=0..7.

T_0 == 1, so the k=0 weight slice folds into an effective bias on the host:
bias_eff[o] = bias[o] + sum_i W[o,i,0]. The device contracts over the
remaining 7*1024 = 7168 (i,k) pairs.

Sharding: data-parallel over batch. Each of the 8 cores takes 512 batch
rows; every core holds the full weights. Per core this is a
[7168 x 512] basis (built on-chip from x) against [7168 x 1024] weights,
accumulated as out.T tiles [128(o) x 512(b)] across 8 PSUM banks with
fp32r matmuls (full PE rate at free-dim 512).
"""

import sys

sys.path.insert(0, "/opt/trn_rl_repo")

import numpy as np

import concourse.bass as bass  # noqa: F401  (engine types come via bacc)
import concourse.mybir as mybir
from concourse import bacc
from concourse.bass_utils import run_bass_kernel_spmd
from concourse.tile import TileContext

P = 128
N_CORES = 8
BATCH = 4096
B_CORE = BATCH // N_CORES  # 512
IN_F = 1024
OUT_F = 1024
KORD = 7  # Chebyshev T_1..T_7 (T_0 folded into bias)
N_ITILES = IN_F // P  # 8
N_OTILES = OUT_F // P  # 8
NSTEPS = N_ITILES * KORD  # 56 contraction steps of K=128

F32 = mybir.dt.float32
F32R = mybir.dt.float32r
ACT_COPY = mybir.ActivationFunctionType.Copy
ACT_TANH = mybir.ActivationFunctionType.Tanh
MULT = mybir.AluOpType.mult

_NC_CACHE = None


def _build():
    """Build + compile the single-core Bass program (SPMD across 8 cores)."""
    global _NC_CACHE
    if _NC_CACHE is not None:
        return _NC_CACHE

    nc = bacc.Bacc("TRN2", target_bir_lowering=False, debug=False)

    # xT[i, b] = x[b, i] for this core's batch slice.
    xT = nc.declare_dram_parameter("xT", [IN_F, B_CORE], F32, isOutput=False)
    # wT[it, k', p, o] = weights[o, it*128+p, k'+1]  (fp32 bits, fp32r view).
    wT = nc.declare_dram_parameter(
        "wT", [N_ITILES, KORD, P, OUT_F], F32R, isOutput=False
    )
    # biasT[p, ot] = bias_eff[ot*128 + p]
    biasT = nc.declare_dram_parameter("biasT", [P, N_OTILES], F32, isOutput=False)
    outT = nc.declare_dram_parameter("outT", [OUT_F, B_CORE], F32, isOutput=True)

    with TileContext(nc) as tc:
        with (
            tc.tile_pool(name="basis", bufs=1) as basis_pool,
            tc.tile_pool(name="chain", bufs=8) as chain_pool,
            tc.tile_pool(name="tmp", bufs=3) as tmp_pool,
            tc.tile_pool(name="raw", bufs=2) as raw_pool,
            tc.tile_pool(name="w", bufs=8) as w_pool,
            tc.tile_pool(name="osb", bufs=3) as osb_pool,
            tc.tile_pool(name="misc", bufs=1) as misc_pool,
            tc.tile_pool(name="psum", bufs=1, space="PSUM") as psum_pool,
        ):
            bias_sb = misc_pool.tile([P, N_OTILES], F32, name="bias_sb")
            nc.sync.dma_start(out=bias_sb, in_=biasT[:, :])

            # ---- Chebyshev basis: chain in fp32, fp32r copies for the PE ----
            # basis_r[it][j] = T_{j+1}(tanh(xT tile it)) as [128, 512] fp32r
            basis_r = []
            for it in range(N_ITILES):
                traw = raw_pool.tile([P, B_CORE], F32, tag="traw")
                nc.sync.dma_start(out=traw, in_=xT[it * P : (it + 1) * P, :])
                t = chain_pool.tile([P, B_CORE], F32, tag="chain")
                nc.scalar.activation(t, traw, ACT_TANH)

                tiles_r = []
                t1r = basis_pool.tile([P, B_CORE], F32R, name=f"b_{it}_0")
                nc.scalar.activation(t1r, t, ACT_COPY)
                tiles_r.append(t1r)

                prev, prev2 = t, None
                for k in range(2, KORD + 1):
                    tmp = tmp_pool.tile([P, B_CORE], F32, tag="tmp")
                    # tmp = (t * 2) * T_{k-1}
                    nc.vector.scalar_tensor_tensor(
                        out=tmp, in0=t, scalar=2.0, in1=prev, op0=MULT, op1=MULT
                    )
                    cur = chain_pool.tile([P, B_CORE], F32, tag="chain")
                    if k == 2:
                        nc.vector.tensor_scalar_sub(cur, tmp, 1.0)
                    else:
                        nc.vector.tensor_sub(cur, tmp, prev2)
                    ckr = basis_pool.tile([P, B_CORE], F32R, name=f"b_{it}_{k - 1}")
                    nc.scalar.activation(ckr, cur, ACT_COPY)
                    tiles_r.append(ckr)
                    prev2, prev = prev, cur
                basis_r.append(tiles_r)

            # ---- Matmul accumulation: out.T[ot] += w_s[:, ot].T @ basis_s ----
            psums = [
                psum_pool.tile([P, B_CORE], F32, name=f"ps_{ot}")
                for ot in range(N_OTILES)
            ]
            HALF = OUT_F // 2
            s = 0
            for it in range(N_ITILES):
                for k in range(KORD):
                    # split the weight fetch so the first 4 matmuls can
                    # start as soon as half the step's weights land
                    wa = w_pool.tile([P, HALF], F32R, tag="wa")
                    nc.sync.dma_start(out=wa, in_=wT[it, k, :, :HALF])
                    wb = w_pool.tile([P, HALF], F32R, tag="wb")
                    nc.sync.dma_start(out=wb, in_=wT[it, k, :, HALF:])
                    rhs = basis_r[it][k]
                    for ot in range(N_OTILES):
                        wt = wa if ot < 4 else wb
                        col = (ot % 4) * P
                        nc.tensor.matmul(
                            psums[ot],
                            lhsT=wt[:, col : col + P],
                            rhs=rhs,
                            start=(s == 0),
                            stop=(s == NSTEPS - 1),
                        )
                    s += 1

            # ---- bias add + store ----
            for ot in range(N_OTILES):
                osb = osb_pool.tile([P, B_CORE], F32, tag="osb")
                nc.scalar.activation(
                    osb,
                    psums[ot],
                    mybir.ActivationFunctionType.Identity,
                    bias=bias_sb[:, ot : ot + 1],
                    scale=1.0,
                )
                nc.sync.dma_start(out=outT[ot * P : (ot + 1) * P, :], in_=osb)

    nc.compile()
    _NC_CACHE = nc
    return _NC_CACHE


def _prep_inputs(x, weights, bias_param):
    x = np.asarray(x, dtype=np.float32)
    weights = np.asarray(weights, dtype=np.float32)
    bias_param = np.asarray(bias_param, dtype=np.float32)

    # [o, i, k] -> [it, k'=k-1, p, o], contiguous
    w4 = weights.transpose(1, 2, 0)[:, 1:, :]  # [i, 7, o]
    w4 = np.ascontiguousarray(
        w4.reshape(N_ITILES, P, KORD, OUT_F).transpose(0, 2, 1, 3)
    )

    bias_eff = bias_param + weights[:, :, 0].sum(axis=1)  # T_0 == 1 fold
    bias_t = np.ascontiguousarray(bias_eff.reshape(N_OTILES, P).T)  # [128, 8]

    in_maps = []
    for c in range(N_CORES):
        x_c = np.ascontiguousarray(x[c * B_CORE : (c + 1) * B_CORE].T)  # [1024, 512]
        in_maps.append({"xT": x_c, "wT": w4, "biasT": bias_t})
    return in_maps


def _run(x, weights, bias_param, **spmd_kwargs):
    nc = _build()
    in_maps = _prep_inputs(x, weights, bias_param)
    res = run_bass_kernel_spmd(nc, in_maps, core_ids=list(range(N_CORES)), **spmd_kwargs)
    out = np.empty((BATCH, OUT_F), dtype=np.float32)
    for c in range(N_CORES):
        out[c * B_CORE : (c + 1) * B_CORE] = res.results[c]["outT"].T
    return out, res


def kernel(x, weights, bias_param):
    out, _ = _run(x, weights, bias_param)
    return out

